# revision 1
# baseline (speedup 1.0000x reference)
"""Context-aware attention kernel for 8 Trainium2 NeuronCores.

Reference computation (B=128, LQ=32, LK=128, D=1024, H=16, DK=64):
  K_cat = concat(K_seq, Q_seq) on the sequence axis  -> [B, 160, D]
  Qh/Kh/Vh = per-head projections, custom exp-based masked attention
  out[b,q,:] = (sum_k mexp[q,k] Vh[k,:]) / (sum_k mexp[q,k] + 1e-8)
  with mexp = exp(QK^T/8) * mask.

Sharding: data-parallel over batch, 16 batches per core. Inside a core the
work is split into 2 halves of 8 batches; each half is processed as
projection GEMMs (bf16 inputs, fp32 PSUM accumulate) followed by attention.
Scores are computed transposed ([k, q] layout) so that the context matmul
takes the masked-exp tile directly as the stationary operand — no on-chip
transposes anywhere (X^T and W^T are prepared on the host). V carries an
extra all-ones column per head so the normalization denominator falls out
of the context matmul for free.
"""

import os
import sys

for _p in ("/opt/trn_rl_repo", "/root/.axon_site/_ro/trn_rl_repo"):
    if os.path.isdir(_p) and _p not in sys.path:
        sys.path.insert(0, _p)

import numpy as np
import ml_dtypes

import concourse.bacc as bacc
import concourse.mybir as mybir
import concourse.tile as tile
from concourse.bass_utils import run_bass_kernel_spmd

B, LQ, LK, D, H, DK = 128, 32, 128, 1024, 16, 64
L = LK + LQ              # 160 keys per batch after concat
NCORES = 8
NB = B // NCORES         # 16 batches per core
NHALF = 2
BH = NB // NHALF         # 8 batches per half
QUAD = 4                 # batches packed into one 128-partition group
NQ_H = BH // QUAD        # quads per half
HC = DK + 1              # per-head V columns incl. the ones column
SCALE = 1.0 / np.sqrt(float(DK))

BF = mybir.dt.bfloat16
F32 = mybir.dt.float32
EXP = mybir.ActivationFunctionType.Exp

_NC = None
_LAST_RESULT = None


def _build():
    nc = bacc.Bacc(
        "TRN2",
        target_bir_lowering=False,
        debug=False,
        enable_asserts=False,
        num_devices=NCORES,
    )
    xt = nc.dram_tensor("xt", [D, NB * L], BF, kind="ExternalInput").ap()
    xvt = nc.dram_tensor("xvt", [D, NB * LK], BF, kind="ExternalInput").ap()
    xqt = nc.dram_tensor("xqt", [D, NB * LQ], BF, kind="ExternalInput").ap()
    wqt = nc.dram_tensor("wqt", [D, D], BF, kind="ExternalInput").ap()
    wkt = nc.dram_tensor("wkt", [D, D], BF, kind="ExternalInput").ap()
    wvt = nc.dram_tensor("wvt", [D, D], BF, kind="ExternalInput").ap()
    bqd = nc.dram_tensor("bq", [8, 128, 1], F32, kind="ExternalInput").ap()
    bkd = nc.dram_tensor("bk", [8, 128, 1], F32, kind="ExternalInput").ap()
    bvd = nc.dram_tensor("bvrow", [1, D], BF, kind="ExternalInput").ap()
    mad = nc.dram_tensor("maska", [NB // QUAD, LK, QUAD * LQ], BF, kind="ExternalInput").ap()
    mbd = nc.dram_tensor("maskb", [NB // QUAD, QUAD * LQ, LQ], BF, kind="ExternalInput").ap()
    outd = nc.dram_tensor("out", [NB * LQ, D], F32, kind="ExternalOutput").ap()

    NQ = NB // QUAD  # 4 quads per core

    from contextlib import ExitStack

    with tile.TileContext(nc) as tc, ExitStack() as st:
        pers = st.enter_context(tc.tile_pool(name="pers", bufs=1))
        xtp = st.enter_context(tc.tile_pool(name="xtp", bufs=16))
        khtp = st.enter_context(tc.tile_pool(name="khtp", bufs=3))
        qhtp = st.enter_context(tc.tile_pool(name="qhtp", bufs=3))
        vhap = st.enter_context(tc.tile_pool(name="vhap", bufs=12))
        vhbp = st.enter_context(tc.tile_pool(name="vhbp", bufs=3))
        mapl = st.enter_context(tc.tile_pool(name="mapl", bufs=3))
        mbpl = st.enter_context(tc.tile_pool(name="mbpl", bufs=3))
        meap = st.enter_context(tc.tile_pool(name="meap", bufs=4))
        mebp = st.enter_context(tc.tile_pool(name="mebp", bufs=4))
        outp = st.enter_context(tc.tile_pool(name="outp", bufs=3))
        mscp = st.enter_context(tc.tile_pool(name="mscp", bufs=4))
        pproj = st.enter_context(tc.tile_pool(name="pproj", bufs=2, space="PSUM"))
        psa = st.enter_context(tc.tile_pool(name="psa", bufs=2, space="PSUM"))
        psb = st.enter_context(tc.tile_pool(name="psb", bufs=2, space="PSUM"))
        pctx = st.enter_context(tc.tile_pool(name="pctx", bufs=2, space="PSUM"))

        # ---- tiny constant DMAs first (they unblock the PE stream) -----
        bq_sb, bk_sb = [], []
        for o in range(8):
            t = pers.tile([128, 1], F32, name=f"bqs{o}", tag=f"bqs{o}")
            nc.sync.dma_start(t[:], bqd[o])
            bq_sb.append(t)
            t = pers.tile([128, 1], F32, name=f"bks{o}", tag=f"bks{o}")
            nc.sync.dma_start(t[:], bkd[o])
            bk_sb.append(t)
        ones1 = pers.tile([1, 128], BF, tag="ones1")
        nc.vector.memset(ones1[:], 1.0)
        bvr = pers.tile([1, D], BF, tag="bvr")
        nc.sync.dma_start(bvr[:], bvd[:])
        bvb = pers.tile([128, D], F32, tag="bvb")
        bvb_v = bvb.rearrange("p (h c) -> p h c", c=DK)

        # ---- input tiles, DMA'd one quad ahead -------------------------
        xin_cache = {}

        def _quad_inputs(gq):
            if gq in xin_cache:
                return xin_cache[gq]
            c0 = gq * QUAD * L
            cv0 = gq * QUAD * LK
            cq0 = gq * QUAD * LQ
            xt_sb = [xtp.tile([128, QUAD * L], BF, name="xt", tag="xt") for _ in range(8)]
            xq_sb = [xtp.tile([128, QUAD * LQ], BF, name="xq", tag="xq", bufs=24) for _ in range(8)]
            xv_sb = [xtp.tile([128, QUAD * LK], BF, name="xv", tag="xv") for _ in range(8)]
            for d in range(8):
                nc.sync.dma_start(xt_sb[d][:], xt[d * 128 : (d + 1) * 128, c0 : c0 + QUAD * L])
            for d in range(8):
                nc.sync.dma_start(xq_sb[d][:], xqt[d * 128 : (d + 1) * 128, cq0 : cq0 + QUAD * LQ])
            for d in range(8):
                nc.sync.dma_start(xv_sb[d][:], xvt[d * 128 : (d + 1) * 128, cv0 : cv0 + QUAD * LK])
            xin_cache[gq] = (xt_sb, xq_sb, xv_sb)
            return xin_cache[gq]

        # ---- weights: wk, then quad-0 inputs, then wq/wv ---------------
        wq_sb, wk_sb, wv_sb = [], [], []
        for lst, nm in ((wk_sb, "wk"), (wq_sb, "wq"), (wv_sb, "wv")):
            for d in range(8):
                lst.append(pers.tile([128, D], BF, name=f"{nm}{d}", tag=f"{nm}{d}"))
        for d in range(8):
            nc.sync.dma_start(wk_sb[d][:], wkt[d * 128 : (d + 1) * 128, :])
        _quad_inputs(0)
        for lst, src in ((wq_sb, wqt), (wv_sb, wvt)):
            for d in range(8):
                nc.sync.dma_start(lst[d][:], src[d * 128 : (d + 1) * 128, :])

        warm_refs = {}

        def _filler():
            # dense dummy matmul group: keeps HAM un-throttled through
            # attention-only stretches (array-sparse small matmuls)
            ps = pproj.tile([128, 512], F32, name="fill", tag="proj")
            for i in range(3):
                nc.tensor.matmul(ps[:], warm_refs["a"][:], warm_refs["b"][:],
                                 start=(i == 0), stop=(i == 2))
            nc.vector.tensor_copy(warm_refs["o"][:], ps[:])

        def _emit_warmup_bvb():
            # Full-array warm-up matmuls on memset data: keeps the PE busy
            # (and HAM un-throttled) while the weight/input DMAs land.
            # K=1 ones-matmuls don't count as PE activity for HAM, so the
            # warm-up uses real 128x128 stationary tiles.
            wma = pers.tile([128, 128], BF, tag="wma")
            nc.vector.memset(wma[:], 1.0 / 128.0)
            wmb = pers.tile([128, 512], BF, tag="wmb")
            nc.vector.memset(wmb[:], 1.0)
            wmo = pers.tile([128, 512], BF, tag="wmo")
            for g in range(10):
                ps = pproj.tile([128, 512], F32, name="warm", tag="proj")
                for i in range(8):
                    nc.tensor.matmul(ps[:], wma[:], wmb[:], start=(i == 0), stop=(i == 7))
                nc.vector.tensor_copy(wmo[:], ps[:])
            box_w = {"a": wma, "b": wmb, "o": wmo}
            warm_refs.update(box_w)
            # broadcast bv across partitions via K=1 matmuls with ones
            for oc in range(2):
                ps = pproj.tile([128, 512], F32, name="warm2", tag="proj")
                nc.tensor.matmul(ps[:], ones1[:], bvr[:, oc * 512 : (oc + 1) * 512],
                                 start=True, stop=True)
                nc.vector.tensor_copy(bvb[:, oc * 512 : (oc + 1) * 512], ps[:])

        # ---- per-quad state --------------------------------------------
        ST = {}

        def _quad_state(gq):
            if gq in ST:
                return ST[gq]
            s = {
                "kht": [khtp.tile([128, QUAD * L], BF, name=f"kht{o}", tag=f"kht{o}") for o in range(8)],
                "qht": [qhtp.tile([128, QUAD * LQ], BF, name=f"qht{o}", tag=f"qht{o}") for o in range(8)],
                "vha": {}, "vhb": None, "ma": None, "mb": None,
            }
            t = mapl.tile([LK, QUAD * LQ], BF, name="ma", tag="ma")
            nc.sync.dma_start(t[:], mad[gq])
            s["ma"] = t
            t = mbpl.tile([QUAD * LQ, LQ], BF, name="mb", tag="mb")
            nc.sync.dma_start(t[:], mbd[gq])
            s["mb"] = t
            ST[gq] = s
            return s

        # ---- projection work units (one PSUM group each) ---------------
        def proj_units(gq):
            s = _quad_state(gq)
            xt_sb, xq_sb, xv_sb = _quad_inputs(gq)
            units = []

            def k_unit(o, sub):
                def run():
                    ps = pproj.tile([128, 512], F32, name="kps", tag="proj")
                    for d in range(8):
                        nc.tensor.matmul(
                            ps[:, 0:320],
                            wk_sb[d][:, o * 128 : (o + 1) * 128],
                            xt_sb[d][:, sub * 320 : (sub + 1) * 320],
                            start=(d == 0), stop=(d == 7),
                        )
                    nc.vector.tensor_scalar_add(
                        s["kht"][o][:, sub * 320 : (sub + 1) * 320],
                        ps[:, 0:320], bk_sb[o][:],
                    )
                return run

            def q_unit(o):
                def run():
                    ps = pproj.tile([128, 512], F32, name="qps", tag="proj")
                    for d in range(8):
                        nc.tensor.matmul(
                            ps[:, 0 : QUAD * LQ],
                            wq_sb[d][:, o * 128 : (o + 1) * 128],
                            xq_sb[d][:], start=(d == 0), stop=(d == 7),
                        )
                    nc.vector.tensor_scalar_add(
                        s["qht"][o][:], ps[:, 0 : QUAD * LQ], bq_sb[o][:]
                    )
                return run

            def va_unit(j, oc):
                def run():
                    gb = gq * QUAD + j
                    if oc == 0:
                        va = vhap.tile([128, H * HC], BF, name="vha", tag="vha")
                        vav = va.rearrange("p (h c) -> p h c", c=HC)
                        nc.vector.memset(vav[:, :, DK : DK + 1], 1.0)
                        s["vha"][gb] = va
                    va = s["vha"][gb]
                    vav = va.rearrange("p (h c) -> p h c", c=HC)
                    ps = pproj.tile([128, 512], F32, name="vps", tag="proj")
                    for d in range(8):
                        nc.tensor.matmul(
                            ps[:],
                            xv_sb[d][:, j * LK : (j + 1) * LK],
                            wv_sb[d][:, oc * 512 : (oc + 1) * 512],
                            start=(d == 0), stop=(d == 7),
                        )
                    nc.vector.tensor_add(
                        vav[:, oc * 8 : (oc + 1) * 8, 0:DK],
                        ps[:].rearrange("p (h c) -> p h c", c=DK),
                        bvb_v[:, oc * 8 : (oc + 1) * 8, :],
                    )
                return run

            def vb_unit(oc):
                def run():
                    if oc == 0:
                        vb = vhbp.tile([128, H * HC], BF, name="vhb", tag="vhb")
                        vbv = vb.rearrange("p (h c) -> p h c", c=HC)
                        nc.vector.memset(vbv[:, :, DK : DK + 1], 1.0)
                        s["vhb"] = vb
                    vb = s["vhb"]
                    vbv = vb.rearrange("p (h c) -> p h c", c=HC)
                    ps = pproj.tile([128, 512], F32, name="vbps", tag="proj")
                    for d in range(8):
                        nc.tensor.matmul(
                            ps[:], xq_sb[d][:],
                            wv_sb[d][:, oc * 512 : (oc + 1) * 512],
                            start=(d == 0), stop=(d == 7),
                        )
                    nc.vector.tensor_add(
                        vbv[:, oc * 8 : (oc + 1) * 8, 0:DK],
                        ps[:].rearrange("p (h c) -> p h c", c=DK),
                        bvb_v[:, oc * 8 : (oc + 1) * 8, :],
                    )
                return run

            for o in range(8):
                for sub in range(2):
                    units.append(k_unit(o, sub))
            for o in range(8):
                units.append(q_unit(o))
            for j in range(QUAD):
                for oc in range(2):
                    units.append(va_unit(j, oc))
            for oc in range(2):
                units.append(vb_unit(oc))
            return units

        # ---- attention work units (software-pipelined over heads) ------
        def attn_units(gq):
            s = _quad_state(gq)
            box = {"pend": {}, "outq": None}

            def scores(h):
                def run():
                    if box["outq"] is None:
                        box["outq"] = outp.tile([128, D], F32, name="outq", tag="outq")
                    ot, h2 = h // 2, 64 * (h % 2)
                    sa4 = psa.tile([LK, QUAD * LQ], F32, name="sa", tag="sa")
                    sb4 = psb.tile([QUAD * LQ, LQ], F32, name="sb", tag="sb")
                    for j in range(QUAD):
                        nc.tensor.matmul(
                            sa4[:, 32 * j : 32 * (j + 1)],
                            s["kht"][ot][h2 : h2 + 64, j * L : j * L + LK],
                            s["qht"][ot][h2 : h2 + 64, j * LQ : (j + 1) * LQ],
                            start=True, stop=True, tile_position=(h2, 0),
                        )
                        nc.tensor.matmul(
                            sb4[32 * j : 32 * (j + 1), :],
                            s["kht"][ot][h2 : h2 + 64, j * L + LK : (j + 1) * L],
                            s["qht"][ot][h2 : h2 + 64, j * LQ : (j + 1) * LQ],
                            start=True, stop=True, tile_position=(h2, 32 * j),
                        )
                    mea4 = meap.tile([LK, QUAD * LQ], BF, name="mea", tag="mea")
                    nc.scalar.activation(mea4[:], sa4[:], EXP, scale=SCALE)
                    nc.vector.tensor_mul(mea4[:], mea4[:], s["ma"][:])
                    meb = mebp.tile([QUAD * LQ, LQ], BF, name="meb", tag="meb")
                    nc.scalar.activation(meb[:], sb4[:], EXP, scale=SCALE)
                    nc.vector.tensor_mul(meb[:], meb[:], s["mb"][:])
                    box["pend"][h] = (mea4, meb)
                return run

            def ctx(h):
                def run():
                    mea4, meb = box["pend"].pop(h)
                    ctxp = pctx.tile([128, HC], F32, name="ctx", tag="ctx")
                    for j in range(QUAD):
                        gb = gq * QUAD + j
                        nc.tensor.matmul(
                            ctxp[32 * j : 32 * (j + 1), :],
                            mea4[:, 32 * j : 32 * (j + 1)],
                            s["vha"][gb][:, h * HC : (h + 1) * HC],
                            start=True, stop=False, tile_position=(0, 32 * j),
                        )
                        nc.tensor.matmul(
                            ctxp[32 * j : 32 * (j + 1), :],
                            meb[32 * j : 32 * (j + 1), :],
                            s["vhb"][32 * j : 32 * (j + 1), h * HC : (h + 1) * HC],
                            start=False, stop=True, tile_position=(32 * j, 32 * j),
                        )
                    r = mscp.tile([128, 1], F32, name="r", tag="r")
                    nc.vector.tensor_scalar_add(r[:], ctxp[:, DK : DK + 1], 1e-8)
                    nc.vector.reciprocal(r[:], r[:])
                    nc.vector.tensor_scalar_mul(
                        box["outq"][:, h * DK : (h + 1) * DK], ctxp[:, 0:DK], r[:]
                    )
                    if h == H - 1:
                        nc.sync.dma_start(
                            outd[gq * 128 : (gq + 1) * 128, :], box["outq"][:]
                        )
                return run

            us = []
            for h in range(H + 1):
                if h < H:
                    us.append(scores(h))
                if h > 0:
                    us.append(ctx(h - 1))
            return us

        def _interleave(a, b, pad_every=3):
            na, nb = len(a), len(b)
            ia = ib = 0
            since_b = 0
            while ia < na or ib < nb:
                if ib >= nb or (ia < na and ia * nb <= ib * na):
                    a[ia]()
                    ia += 1
                    since_b += 1
                    if ib >= nb and since_b >= pad_every and ia < na:
                        _filler()
                        since_b = 0
                else:
                    b[ib]()
                    ib += 1
                    since_b = 0

        # ---- emission schedule -----------------------------------------
        _emit_warmup_bvb()
        for u in proj_units(0):
            u()
        for gq in range(NQ):
            if gq + 2 < NQ:
                _quad_inputs(gq + 2)
                _quad_state(gq + 2)
            nxt = proj_units(gq + 1) if gq + 1 < NQ else []
            _interleave(attn_units(gq), nxt)

    nc.compile()
    return nc


def _get_nc():
    global _NC
    if _NC is None:
        _NC = _build()
    return _NC


def kernel(**inputs):
    global _LAST_RESULT
    Q_seq = np.asarray(inputs["Q_seq"], dtype=np.float32)
    K_seq = np.asarray(inputs["K_seq"], dtype=np.float32)
    V_seq = np.asarray(inputs["V_seq"], dtype=np.float32)
    tm = np.asarray(inputs["title_mask"], dtype=np.float32)
    bm = np.asarray(inputs["body_mask"], dtype=np.float32)
    Wq = np.asarray(inputs["Wq"], dtype=np.float32)
    Wk = np.asarray(inputs["Wk"], dtype=np.float32)
    Wv = np.asarray(inputs["Wv"], dtype=np.float32)
    bq = np.asarray(inputs["bq"], dtype=np.float32)
    bk = np.asarray(inputs["bk"], dtype=np.float32)
    bv = np.asarray(inputs["bv"], dtype=np.float32)

    bf = ml_dtypes.bfloat16
    # K_cat = concat(K_seq, Q_seq); V_cat = concat(V_seq, Q_seq). The V
    # projection of the shared Q_seq rows reuses xt's query columns, so
    # xvt only carries the V_seq part.
    Xk = np.concatenate([K_seq, Q_seq], axis=1)  # [B, L, D]

    wqt = np.ascontiguousarray(Wq.T).astype(bf)
    wkt = np.ascontiguousarray(Wk.T).astype(bf)
    wvt = np.ascontiguousarray(Wv.T).astype(bf)

    maska_bt = (bm * tm[:, :, None]).transpose(0, 2, 1)  # [B,128,32]
    maska = np.ascontiguousarray(
        maska_bt.reshape(B // QUAD, QUAD, LK, LQ).transpose(0, 2, 1, 3).reshape(B // QUAD, LK, QUAD * LQ)
    ).astype(bf)  # [B/4, 128, 4*32]
    maskb = (tm[:, :, None] * tm[:, None, :]).astype(bf)  # [B, 32(i), 32(q)]

    nc = _get_nc()
    in_maps = []
    for c in range(NCORES):
        sl = slice(c * NB, (c + 1) * NB)
        XT = np.ascontiguousarray(Xk[sl].reshape(NB * L, D).T).astype(bf)
        XVT = np.ascontiguousarray(V_seq[sl].reshape(NB * LK, D).T).astype(bf)
        XQT = np.ascontiguousarray(Q_seq[sl].reshape(NB * LQ, D).T).astype(bf)
        in_maps.append({
            "xt": XT,
            "xvt": XVT,
            "xqt": XQT,
            "wqt": wqt, "wkt": wkt, "wvt": wvt,
            "bq": np.ascontiguousarray(bq.reshape(8, 128, 1)),
            "bk": np.ascontiguousarray(bk.reshape(8, 128, 1)),
            "bvrow": np.ascontiguousarray(bv.reshape(1, D)).astype(bf),
            "maska": np.ascontiguousarray(maska[c * NB // QUAD : (c + 1) * NB // QUAD]),
            "maskb": np.ascontiguousarray(maskb[sl].reshape(NB // QUAD, QUAD * LQ, LQ)),
        })

    res = run_bass_kernel_spmd(nc, in_maps, core_ids=list(range(NCORES)))
    _LAST_RESULT = res
    out = np.concatenate(
        [res.results[c]["out"].reshape(NB, LQ, D) for c in range(NCORES)], axis=0
    )
    return np.ascontiguousarray(out.astype(np.float32))



# revision 10
# speedup vs baseline: 1.2846x; 1.2846x over previous
"""Context-aware attention kernel for 8 Trainium2 NeuronCores.

Reference computation (B=128, LQ=32, LK=128, D=1024, H=16, DK=64):
  K_cat = concat(K_seq, Q_seq) on the sequence axis  -> [B, 160, D]
  Qh/Kh/Vh = per-head projections, custom exp-based masked attention
  out[b,q,:] = (sum_k mexp[q,k] Vh[k,:]) / (sum_k mexp[q,k] + 1e-8)
  with mexp = exp(QK^T/8) * mask.

Sharding: data-parallel over batch, 16 batches per core, processed as 4
quads of 4 batches. Projections are dense GEMMs (bf16 in, fp32 PSUM).

Attention packs heads to keep matmul streams long:
  * scores: stationary = Kh o-block [128 dk(2 heads), keys], moving = a
    block-diagonal zero-padded Q tile [128, 64] carrying BOTH heads'
    queries -> one N=64 matmul per (batch, head-pair) instead of four
    N=32 ones.  Output lands [keys, (head, q)].
  * context: stationary = masked-exp scores [keys, 64 (2 heads x 32 q)],
    moving = [V_h0|V_h1|ones] [keys, 129] -> N=129 matmuls.  The shared
    ones column yields every (head, q) denominator in the PSUM tile for
    free.  Four batches of one head-pair share a single PSUM bank via
    tile-position slots ([128, 258] = 4 slots of [64, 129]).
  * epilogue: reciprocal on DVE, per-head Copy-with-scale activations on
    the scalar engine gather the diagonal blocks into the output tile.
"""

import os
import sys

for _p in ("/opt/trn_rl_repo", "/root/.axon_site/_ro/trn_rl_repo"):
    if os.path.isdir(_p) and _p not in sys.path:
        sys.path.insert(0, _p)

import numpy as np
import ml_dtypes

import concourse.bacc as bacc
import concourse.mybir as mybir
import concourse.tile as tile
from concourse.bass_utils import run_bass_kernel_spmd

B, LQ, LK, D, H, DK = 128, 32, 128, 1024, 16, 64
L = LK + LQ              # 160 keys per batch after concat
NCORES = 8
NB = B // NCORES         # 16 batches per core
QUAD = 4                 # batches packed into one 128-partition group
NQ = NB // QUAD          # 4 quads per core
NHG = 4                  # head groups of 4 heads (score granularity)
HPW = 2 * DK + 1         # 129 V columns per head pair incl the ones col
SCALE = 1.0 / np.sqrt(float(DK))

BF = mybir.dt.bfloat16
F32 = mybir.dt.float32
EXP = mybir.ActivationFunctionType.Exp
COPY = mybir.ActivationFunctionType.Copy

_NC = None
_LAST_RESULT = None


def _build():
    nc = bacc.Bacc(
        "TRN2",
        target_bir_lowering=False,
        debug=False,
        enable_asserts=False,
        num_devices=NCORES,
    )
    xt = nc.dram_tensor("xt", [D, NB * L], BF, kind="ExternalInput").ap()
    xvt = nc.dram_tensor("xvt", [D, NB * LK], BF, kind="ExternalInput").ap()
    xqt = nc.dram_tensor("xqt", [D, NB * LQ], BF, kind="ExternalInput").ap()
    wqt = nc.dram_tensor("wqt", [D, D], BF, kind="ExternalInput").ap()
    wkt = nc.dram_tensor("wkt", [D, D], BF, kind="ExternalInput").ap()
    wvt = nc.dram_tensor("wvt", [D, D], BF, kind="ExternalInput").ap()
    bqd = nc.dram_tensor("bq", [8, 128, 1], F32, kind="ExternalInput").ap()
    bkd = nc.dram_tensor("bk", [8, 128, 1], F32, kind="ExternalInput").ap()
    bvd = nc.dram_tensor("bvrow", [1, D], BF, kind="ExternalInput").ap()
    mad = nc.dram_tensor("maska", [NQ, LK, QUAD * 4 * LQ], BF, kind="ExternalInput").ap()
    mbd = nc.dram_tensor("maskb", [NQ, QUAD * LQ, 4 * LQ], BF, kind="ExternalInput").ap()
    outd = nc.dram_tensor("out", [NB * LQ, D], F32, kind="ExternalOutput").ap()

    from contextlib import ExitStack

    with tile.TileContext(nc) as tc, ExitStack() as st:
        pers = st.enter_context(tc.tile_pool(name="pers", bufs=1))
        xtp = st.enter_context(tc.tile_pool(name="xtp", bufs=16))
        khtp = st.enter_context(tc.tile_pool(name="khtp", bufs=3))
        meap = st.enter_context(tc.tile_pool(name="meap", bufs=12))
        mebp = st.enter_context(tc.tile_pool(name="mebp", bufs=3))
        mapl = st.enter_context(tc.tile_pool(name="mapl", bufs=3))
        mbpl = st.enter_context(tc.tile_pool(name="mbpl", bufs=3))
        outp = st.enter_context(tc.tile_pool(name="outp", bufs=2))
        mscp = st.enter_context(tc.tile_pool(name="mscp", bufs=8))
        pproj = st.enter_context(tc.tile_pool(name="pproj", bufs=2, space="PSUM"))
        psa = st.enter_context(tc.tile_pool(name="psa", bufs=2, space="PSUM"))
        psb = st.enter_context(tc.tile_pool(name="psb", bufs=2, space="PSUM"))
        pcp = st.enter_context(tc.tile_pool(name="pcp", bufs=2, space="PSUM"))

        # ---- tiny constant DMAs first (they unblock the PE stream) -----
        bq_sb, bk_sb = [], []
        for o in range(8):
            t = pers.tile([128, 1], F32, name=f"bqs{o}", tag=f"bqs{o}")
            nc.sync.dma_start(t[:], bqd[o])
            bq_sb.append(t)
            t = pers.tile([128, 1], F32, name=f"bks{o}", tag=f"bks{o}")
            nc.sync.dma_start(t[:], bkd[o])
            bk_sb.append(t)
        ones1 = pers.tile([1, 128], BF, tag="ones1")
        nc.vector.memset(ones1[:], 1.0)
        bvr = pers.tile([1, D], BF, tag="bvr")
        nc.sync.dma_start(bvr[:], bvd[:])
        bvb = pers.tile([128, D], F32, tag="bvb")

        # ---- persistent packed-Q / packed-V tiles (double-set by quad
        # parity).  qpk zero regions and vpa/vhb ones columns are written
        # exactly once here; the per-quad adds only touch the data regions.
        qpk = [[pers.tile([128, QUAD * 64], BF, name=f"qpk{s}_{o}", tag=f"qpk{s}_{o}") for o in range(8)]
               for s in range(2)]
        vpa = [[pers.tile([128, 8 * HPW], BF, name=f"vpa{s}_{j}", tag=f"vpa{s}_{j}") for j in range(QUAD)]
               for s in range(2)]
        vhb = [pers.tile([128, 8 * HPW], BF, name=f"vhb{s}", tag=f"vhb{s}") for s in range(2)]
        for s in range(2):
            for o in range(8):
                nc.vector.memset(qpk[s][o][:], 0.0)
            for j in range(QUAD):
                nc.vector.memset(vpa[s][j][:], 1.0)
            nc.vector.memset(vhb[s][:], 1.0)

        # ---- input tiles, DMA'd one quad ahead -------------------------
        xin_cache = {}

        def _quad_inputs(gq):
            if gq in xin_cache:
                return xin_cache[gq]
            c0 = gq * QUAD * L
            cv0 = gq * QUAD * LK
            cq0 = gq * QUAD * LQ
            xt_sb = [xtp.tile([128, QUAD * L], BF, name="xt", tag="xt") for _ in range(8)]
            xq_sb = [xtp.tile([128, QUAD * LQ], BF, name="xq", tag="xq", bufs=24) for _ in range(8)]
            xv_sb = [xtp.tile([128, QUAD * LK], BF, name="xv", tag="xv") for _ in range(8)]
            for d in range(8):
                nc.sync.dma_start(xt_sb[d][:], xt[d * 128 : (d + 1) * 128, c0 : c0 + QUAD * L])
            for d in range(8):
                nc.sync.dma_start(xq_sb[d][:], xqt[d * 128 : (d + 1) * 128, cq0 : cq0 + QUAD * LQ])
            for d in range(8):
                nc.sync.dma_start(xv_sb[d][:], xvt[d * 128 : (d + 1) * 128, cv0 : cv0 + QUAD * LK])
            xin_cache[gq] = (xt_sb, xq_sb, xv_sb)
            return xin_cache[gq]

        # ---- weights: wk, then quad-0 inputs, then wq/wv ---------------
        wq_sb, wk_sb, wv_sb = [], [], []
        for lst, nm in ((wk_sb, "wk"), (wq_sb, "wq"), (wv_sb, "wv")):
            for d in range(8):
                lst.append(pers.tile([128, D], BF, name=f"{nm}{d}", tag=f"{nm}{d}"))
        for d in range(8):
            nc.sync.dma_start(wk_sb[d][:], wkt[d * 128 : (d + 1) * 128, :])
        _quad_inputs(0)
        for lst, src in ((wq_sb, wqt), (wv_sb, wvt)):
            for d in range(8):
                nc.sync.dma_start(lst[d][:], src[d * 128 : (d + 1) * 128, :])

        warm_refs = {}

        def _filler():
            # dense dummy matmul group: keeps HAM un-throttled through
            # attention-only stretches
            ps = pproj.tile([128, 512], F32, name="fill", tag="proj")
            for i in range(3):
                nc.tensor.matmul(ps[:], warm_refs["a"][:], warm_refs["b"][:],
                                 start=(i == 0), stop=(i == 2))
            nc.vector.tensor_copy(warm_refs["o"][:], ps[:])

        def _emit_warmup_bvb():
            # Full-array warm-up matmuls on memset data: keeps the PE busy
            # (and HAM un-throttled) while the weight/input DMAs land.
            wma = pers.tile([128, 128], BF, tag="wma")
            nc.vector.memset(wma[:], 1.0 / 128.0)
            wmb = pers.tile([128, 512], BF, tag="wmb")
            nc.vector.memset(wmb[:], 1.0)
            wmo = pers.tile([128, 512], BF, tag="wmo")
            for g in range(10):
                ps = pproj.tile([128, 512], F32, name="warm", tag="proj")
                for i in range(8):
                    nc.tensor.matmul(ps[:], wma[:], wmb[:], start=(i == 0), stop=(i == 7))
                nc.vector.tensor_copy(wmo[:], ps[:])
            warm_refs.update({"a": wma, "b": wmb, "o": wmo})
            # broadcast bv across partitions via K=1 matmuls with ones
            for oc in range(2):
                ps = pproj.tile([128, 512], F32, name="warm2", tag="proj")
                nc.tensor.matmul(ps[:], ones1[:], bvr[:, oc * 512 : (oc + 1) * 512],
                                 start=True, stop=True)
                nc.vector.tensor_copy(bvb[:, oc * 512 : (oc + 1) * 512], ps[:])

        # ---- per-quad state --------------------------------------------
        ST = {}

        def _quad_state(gq):
            if gq in ST:
                return ST[gq]
            s = {
                "kht": [khtp.tile([128, QUAD * L], BF, name=f"kht{o}", tag=f"kht{o}") for o in range(8)],
                "ma": None, "mb": None,
            }
            t = mapl.tile([LK, QUAD * 4 * LQ], BF, name="ma", tag="ma")
            nc.sync.dma_start(t[:], mad[gq])
            s["ma"] = t
            t = mbpl.tile([QUAD * LQ, 4 * LQ], BF, name="mb", tag="mb")
            nc.sync.dma_start(t[:], mbd[gq])
            s["mb"] = t
            ST[gq] = s
            return s

        # ---- projection work units (one PSUM group each) ---------------
        def proj_units(gq):
            s = _quad_state(gq)
            par = gq % 2
            xt_sb, xq_sb, xv_sb = _quad_inputs(gq)
            units = []

            def k_unit(o, sub):
                def run():
                    ps = pproj.tile([128, 512], F32, name="kps", tag="proj")
                    for d in range(8):
                        nc.tensor.matmul(
                            ps[:, 0:320],
                            wk_sb[d][:, o * 128 : (o + 1) * 128],
                            xt_sb[d][:, sub * 320 : (sub + 1) * 320],
                            start=(d == 0), stop=(d == 7),
                        )
                    nc.vector.tensor_scalar_add(
                        s["kht"][o][:, sub * 320 : (sub + 1) * 320],
                        ps[:, 0:320], bk_sb[o][:],
                    )
                return run

            def q_unit(o):
                def run():
                    ps = pproj.tile([128, 512], F32, name="qps", tag="proj")
                    for d in range(8):
                        nc.tensor.matmul(
                            ps[:, 0 : QUAD * LQ],
                            wq_sb[d][:, o * 128 : (o + 1) * 128],
                            xq_sb[d][:], start=(d == 0), stop=(d == 7),
                        )
                    # scatter into the block-diagonal packed-Q tile:
                    # head-even rows -> col block [0,32), head-odd rows ->
                    # col block [32,64) of each batch's 64-col group.
                    qv = qpk[par][o].rearrange("p (j c) -> p j c", c=64)
                    pv = ps.rearrange("p (j c) -> p j c", c=LQ)
                    nc.vector.tensor_scalar_add(
                        qv[0:64, 0:QUAD, 0:32], pv[0:64, 0:QUAD, :], bq_sb[o][0:64]
                    )
                    nc.vector.tensor_scalar_add(
                        qv[64:128, 0:QUAD, 32:64], pv[64:128, 0:QUAD, :], bq_sb[o][64:128]
                    )
                return run

            def va_unit(j, oc):
                def run():
                    ps = pproj.tile([128, 512], F32, name="vps", tag="proj")
                    for d in range(8):
                        nc.tensor.matmul(
                            ps[:],
                            xv_sb[d][:, j * LK : (j + 1) * LK],
                            wv_sb[d][:, oc * 512 : (oc + 1) * 512],
                            start=(d == 0), stop=(d == 7),
                        )
                    for t in range(4):
                        hp = oc * 4 + t
                        nc.vector.tensor_add(
                            vpa[par][j][:, hp * HPW : hp * HPW + 128],
                            ps[:, t * 128 : (t + 1) * 128],
                            bvb[:, hp * 128 : (hp + 1) * 128],
                        )
                return run

            def vb_unit(oc):
                def run():
                    ps = pproj.tile([128, 512], F32, name="vbps", tag="proj")
                    for d in range(8):
                        nc.tensor.matmul(
                            ps[:], xq_sb[d][:],
                            wv_sb[d][:, oc * 512 : (oc + 1) * 512],
                            start=(d == 0), stop=(d == 7),
                        )
                    for t in range(4):
                        hp = oc * 4 + t
                        nc.vector.tensor_add(
                            vhb[par][:, hp * HPW : hp * HPW + 128],
                            ps[:, t * 128 : (t + 1) * 128],
                            bvb[:, hp * 128 : (hp + 1) * 128],
                        )
                return run

            for o in range(8):
                for sub in range(2):
                    units.append(k_unit(o, sub))
            for o in range(8):
                units.append(q_unit(o))
            for j in range(QUAD):
                for oc in range(2):
                    units.append(va_unit(j, oc))
            for oc in range(2):
                units.append(vb_unit(oc))
            return units

        # ---- attention work units (software-pipelined over head groups) -
        def attn_units(gq):
            s = _quad_state(gq)
            par = gq % 2
            box = {"psa": {}, "psb": {}, "mea": {}, "meb": {}, "psc": {}, "outq": None}

            def a_unit(hg):
                def run():
                    if box["outq"] is None:
                        box["outq"] = outp.tile([128, D], F32, name="outq", tag="outq")
                    pa = psa.tile([128, 512], F32, name="sa", tag="sa")
                    for j in range(QUAD):
                        for p in range(2):
                            o = 2 * hg + p
                            nc.tensor.matmul(
                                pa[:, j * 128 + p * 64 : j * 128 + (p + 1) * 64],
                                s["kht"][o][:, j * L : j * L + LK],
                                qpk[par][o][:, j * 64 : (j + 1) * 64],
                                start=True, stop=True,
                            )
                    box["psa"][hg] = pa
                return run

            def b_unit(hg):
                def run():
                    pb = psb.tile([128, 128], F32, name="sb", tag="sb")
                    for j in range(QUAD):
                        for p in range(2):
                            o = 2 * hg + p
                            nc.tensor.matmul(
                                pb[j * 32 : (j + 1) * 32, p * 64 : (p + 1) * 64],
                                s["kht"][o][:, j * L + LK : (j + 1) * L],
                                qpk[par][o][:, j * 64 : (j + 1) * 64],
                                start=True, stop=True, tile_position=(0, 32 * j),
                            )
                    box["psb"][hg] = pb
                return run

            def e_unit(hg):
                def run():
                    pa = box["psa"].pop(hg)
                    mea = []
                    for j in range(QUAD):
                        m = meap.tile([LK, 4 * LQ], BF, name="mea", tag="mea")
                        nc.scalar.activation(m[:], pa[:, j * 128 : (j + 1) * 128], EXP, scale=SCALE)
                        nc.vector.tensor_mul(m[:], m[:], s["ma"][:, j * 128 : (j + 1) * 128])
                        mea.append(m)
                    box["mea"][hg] = mea
                    pb = box["psb"].pop(hg)
                    meb = mebp.tile([QUAD * LQ, 4 * LQ], BF, name="meb", tag="meb")
                    nc.scalar.activation(meb[:], pb[:], EXP, scale=SCALE)
                    nc.vector.tensor_mul(meb[:], meb[:], s["mb"][:])
                    box["meb"][hg] = meb
                return run

            def c_unit(hg, u):
                # one head pair (heads 4*hg+2*u, 4*hg+2*u+1) x 4 batches,
                # all four batches sharing one PSUM bank via [64,129] slots
                def run():
                    mea = box["mea"][hg]
                    meb = box["meb"][hg]
                    hp = 2 * hg + u
                    pc = pcp.tile([128, 2 * HPW], F32, name="ctx", tag="ctx")
                    for j in range(QUAD):
                        jm, jc = j % 2, j // 2
                        nc.tensor.matmul(
                            pc[64 * jm : 64 * jm + 64, HPW * jc : HPW * (jc + 1)],
                            mea[j][:, u * 64 : (u + 1) * 64],
                            vpa[par][j][:, hp * HPW : (hp + 1) * HPW],
                            start=True, stop=False, tile_position=(0, 64 * jm),
                        )
                        nc.tensor.matmul(
                            pc[64 * jm : 64 * jm + 64, HPW * jc : HPW * (jc + 1)],
                            meb[j * 32 : (j + 1) * 32, u * 64 : (u + 1) * 64],
                            vhb[par][j * 32 : (j + 1) * 32, hp * HPW : (hp + 1) * HPW],
                            start=False, stop=True, tile_position=(32 * j, 64 * jm),
                        )
                    box["psc"][(hg, u)] = pc
                return run

            def d_unit(hg, u):
                def run():
                    pc = box["psc"].pop((hg, u))
                    outq = box["outq"]
                    r = mscp.tile([128, 2], F32, name="r", tag="r")
                    for jc in range(2):
                        nc.vector.tensor_scalar_add(
                            r[:, jc : jc + 1], pc[:, HPW * jc + 128 : HPW * jc + 129], 1e-8
                        )
                    nc.vector.reciprocal(r[:], r[:])
                    for j in range(QUAD):
                        jm, jc = j % 2, j // 2
                        for h2 in range(2):
                            h = 4 * hg + 2 * u + h2
                            nc.scalar.activation(
                                outq[j * 32 : (j + 1) * 32, h * DK : (h + 1) * DK],
                                pc[64 * jm + 32 * h2 : 64 * jm + 32 * h2 + 32,
                                   HPW * jc + 64 * h2 : HPW * jc + 64 * h2 + 64],
                                COPY, scale=r[64 * jm + 32 * h2 : 64 * jm + 32 * h2 + 32, jc : jc + 1],
                            )
                    if (hg, u) == (NHG - 1, 1):
                        nc.sync.dma_start(
                            outd[gq * 128 : (gq + 1) * 128, :], outq[:]
                        )
                return run

            # software pipeline: keep the PE fed (A/B/C) while the scalar
            # and vector engines chew on E/D of earlier head groups.
            us = [a_unit(0), b_unit(0), e_unit(0),
                  a_unit(1), b_unit(1), c_unit(0, 0), c_unit(0, 1), e_unit(1),
                  a_unit(2), b_unit(2), d_unit(0, 0), c_unit(1, 0), d_unit(0, 1),
                  c_unit(1, 1), e_unit(2),
                  a_unit(3), b_unit(3), d_unit(1, 0), c_unit(2, 0), d_unit(1, 1),
                  c_unit(2, 1), e_unit(3),
                  d_unit(2, 0), c_unit(3, 0), d_unit(2, 1), c_unit(3, 1),
                  d_unit(3, 0), d_unit(3, 1)]
            return us

        def _interleave(a, b, pad_every=3):
            na, nb = len(a), len(b)
            ia = ib = 0
            since_b = 0
            while ia < na or ib < nb:
                if ib >= nb or (ia < na and ia * nb <= ib * na):
                    a[ia]()
                    ia += 1
                    since_b += 1
                    if ib >= nb and since_b >= pad_every and ia < na:
                        _filler()
                        since_b = 0
                else:
                    b[ib]()
                    ib += 1
                    since_b = 0

        # ---- emission schedule -----------------------------------------
        _emit_warmup_bvb()
        for u in proj_units(0):
            u()
        for gq in range(NQ):
            if gq + 2 < NQ:
                _quad_inputs(gq + 2)
                _quad_state(gq + 2)
            nxt = proj_units(gq + 1) if gq + 1 < NQ else []
            _interleave(attn_units(gq), nxt)

    nc.compile()
    return nc


def _get_nc():
    global _NC
    if _NC is None:
        _NC = _build()
    return _NC


def kernel(**inputs):
    global _LAST_RESULT
    Q_seq = np.asarray(inputs["Q_seq"], dtype=np.float32)
    K_seq = np.asarray(inputs["K_seq"], dtype=np.float32)
    V_seq = np.asarray(inputs["V_seq"], dtype=np.float32)
    tm = np.asarray(inputs["title_mask"], dtype=np.float32)
    bm = np.asarray(inputs["body_mask"], dtype=np.float32)
    Wq = np.asarray(inputs["Wq"], dtype=np.float32)
    Wk = np.asarray(inputs["Wk"], dtype=np.float32)
    Wv = np.asarray(inputs["Wv"], dtype=np.float32)
    bq = np.asarray(inputs["bq"], dtype=np.float32)
    bk = np.asarray(inputs["bk"], dtype=np.float32)
    bv = np.asarray(inputs["bv"], dtype=np.float32)

    bf = ml_dtypes.bfloat16
    # K_cat = concat(K_seq, Q_seq); V_cat = concat(V_seq, Q_seq). The V
    # projection of the shared Q_seq rows reuses xqt, so xvt only carries
    # the V_seq part.
    Xk = np.concatenate([K_seq, Q_seq], axis=1)  # [B, L, D]

    wqt = np.ascontiguousarray(Wq.T).astype(bf)
    wkt = np.ascontiguousarray(Wk.T).astype(bf)
    wvt = np.ascontiguousarray(Wv.T).astype(bf)

    # body mask [B, 128 keys, 32 q] tiled x4 along cols -> [B, 128, 128]
    maska_bt = (bm * tm[:, :, None]).transpose(0, 2, 1)  # [B,128,32]
    maska4 = np.tile(maska_bt, (1, 1, 4))  # [B, 128, 128]
    # title mask outer product, tiled x4 -> [B, 32, 128]
    maskb = tm[:, :, None] * tm[:, None, :]  # [B, 32(i), 32(q)]
    maskb4 = np.tile(maskb, (1, 1, 4))  # [B, 32, 128]

    nc = _get_nc()
    in_maps = []
    for c in range(NCORES):
        sl = slice(c * NB, (c + 1) * NB)
        XT = np.ascontiguousarray(Xk[sl].reshape(NB * L, D).T).astype(bf)
        XVT = np.ascontiguousarray(V_seq[sl].reshape(NB * LK, D).T).astype(bf)
        XQT = np.ascontiguousarray(Q_seq[sl].reshape(NB * LQ, D).T).astype(bf)
        ma = maska4[sl].reshape(NQ, QUAD, LK, 4 * LQ).transpose(0, 2, 1, 3)
        ma = np.ascontiguousarray(ma.reshape(NQ, LK, QUAD * 4 * LQ)).astype(bf)
        mb = np.ascontiguousarray(maskb4[sl].reshape(NQ, QUAD * LQ, 4 * LQ)).astype(bf)
        in_maps.append({
            "xt": XT,
            "xvt": XVT,
            "xqt": XQT,
            "wqt": wqt, "wkt": wkt, "wvt": wvt,
            "bq": np.ascontiguousarray(bq.reshape(8, 128, 1)),
            "bk": np.ascontiguousarray(bk.reshape(8, 128, 1)),
            "bvrow": np.ascontiguousarray(bv.reshape(1, D)).astype(bf),
            "maska": ma,
            "maskb": mb,
        })

    res = run_bass_kernel_spmd(nc, in_maps, core_ids=list(range(NCORES)))
    _LAST_RESULT = res
    out = np.concatenate(
        [res.results[c]["out"].reshape(NB, LQ, D) for c in range(NCORES)], axis=0
    )
    return np.ascontiguousarray(out.astype(np.float32))


# revision 11
# speedup vs baseline: 1.2997x; 1.0118x over previous
"""Context-aware attention kernel for 8 Trainium2 NeuronCores.

Reference computation (B=128, LQ=32, LK=128, D=1024, H=16, DK=64):
  K_cat = concat(K_seq, Q_seq) on the sequence axis  -> [B, 160, D]
  Qh/Kh/Vh = per-head projections, custom exp-based masked attention
  out[b,q,:] = (sum_k mexp[q,k] Vh[k,:]) / (sum_k mexp[q,k] + 1e-8)
  with mexp = exp(QK^T/8) * mask.

Sharding: data-parallel over batch, 16 batches per core, processed as 4
quads of 4 batches. Projections are dense GEMMs (bf16 in, fp32 PSUM).

Attention packs heads to keep matmul streams long:
  * scores: stationary = Kh o-block [128 dk(2 heads), keys], moving = a
    block-diagonal zero-padded Q tile [128, 64] carrying BOTH heads'
    queries -> one N=64 matmul per (batch, head-pair) instead of four
    N=32 ones.  Output lands [keys, (head, q)].
  * context: stationary = masked-exp scores [keys, 64 (2 heads x 32 q)],
    moving = [V_h0|V_h1|ones] [keys, 129] -> N=129 matmuls.  The shared
    ones column yields every (head, q) denominator in the PSUM tile for
    free.  Four batches of one head-pair share a single PSUM bank via
    tile-position slots ([128, 258] = 4 slots of [64, 129]).
  * epilogue: reciprocal on DVE, per-head Copy-with-scale activations on
    the scalar engine gather the diagonal blocks into the output tile.
"""

import os
import sys

for _p in ("/opt/trn_rl_repo", "/root/.axon_site/_ro/trn_rl_repo"):
    if os.path.isdir(_p) and _p not in sys.path:
        sys.path.insert(0, _p)

import numpy as np
import ml_dtypes

import concourse.bacc as bacc
import concourse.mybir as mybir
import concourse.tile as tile
from concourse.bass_utils import run_bass_kernel_spmd

B, LQ, LK, D, H, DK = 128, 32, 128, 1024, 16, 64
L = LK + LQ              # 160 keys per batch after concat
NCORES = 8
NB = B // NCORES         # 16 batches per core
QUAD = 4                 # batches packed into one 128-partition group
NQ = NB // QUAD          # 4 quads per core
NHG = 4                  # head groups of 4 heads (score granularity)
HPW = 2 * DK + 1         # 129 V columns per head pair incl the ones col
SCALE = 1.0 / np.sqrt(float(DK))

BF = mybir.dt.bfloat16
F32 = mybir.dt.float32
EXP = mybir.ActivationFunctionType.Exp
COPY = mybir.ActivationFunctionType.Copy

_NC = None
_LAST_RESULT = None


def _build():
    nc = bacc.Bacc(
        "TRN2",
        target_bir_lowering=False,
        debug=False,
        enable_asserts=False,
        num_devices=NCORES,
    )
    xt = nc.dram_tensor("xt", [D, NB * L], BF, kind="ExternalInput").ap()
    xvt = nc.dram_tensor("xvt", [D, NB * LK], BF, kind="ExternalInput").ap()
    xqt = nc.dram_tensor("xqt", [D, NB * LQ], BF, kind="ExternalInput").ap()
    wqt = nc.dram_tensor("wqt", [D, D], BF, kind="ExternalInput").ap()
    wkt = nc.dram_tensor("wkt", [D, D], BF, kind="ExternalInput").ap()
    wvt = nc.dram_tensor("wvt", [D, D], BF, kind="ExternalInput").ap()
    bqd = nc.dram_tensor("bq", [8, 128, 1], F32, kind="ExternalInput").ap()
    bkd = nc.dram_tensor("bk", [8, 128, 1], F32, kind="ExternalInput").ap()
    bvd = nc.dram_tensor("bvrow", [1, D], BF, kind="ExternalInput").ap()
    mad = nc.dram_tensor("maska", [NQ, LK, QUAD * 4 * LQ], BF, kind="ExternalInput").ap()
    mbd = nc.dram_tensor("maskb", [NQ, QUAD * LQ, 4 * LQ], BF, kind="ExternalInput").ap()
    outd = nc.dram_tensor("out", [NB * LQ, D], BF, kind="ExternalOutput").ap()

    from contextlib import ExitStack

    with tile.TileContext(nc) as tc, ExitStack() as st:
        pers = st.enter_context(tc.tile_pool(name="pers", bufs=1))
        xtp = st.enter_context(tc.tile_pool(name="xtp", bufs=16))
        khtp = st.enter_context(tc.tile_pool(name="khtp", bufs=3))
        meap = st.enter_context(tc.tile_pool(name="meap", bufs=12))
        mebp = st.enter_context(tc.tile_pool(name="mebp", bufs=3))
        mapl = st.enter_context(tc.tile_pool(name="mapl", bufs=3))
        mbpl = st.enter_context(tc.tile_pool(name="mbpl", bufs=3))
        outp = st.enter_context(tc.tile_pool(name="outp", bufs=2))
        mscp = st.enter_context(tc.tile_pool(name="mscp", bufs=8))
        pproj = st.enter_context(tc.tile_pool(name="pproj", bufs=2, space="PSUM"))
        psa = st.enter_context(tc.tile_pool(name="psa", bufs=2, space="PSUM"))
        psb = st.enter_context(tc.tile_pool(name="psb", bufs=2, space="PSUM"))
        pcp = st.enter_context(tc.tile_pool(name="pcp", bufs=2, space="PSUM"))

        # ---- tiny constant DMAs first (they unblock the PE stream) -----
        bq_sb, bk_sb = [], []
        for o in range(8):
            t = pers.tile([128, 1], F32, name=f"bqs{o}", tag=f"bqs{o}")
            nc.sync.dma_start(t[:], bqd[o])
            bq_sb.append(t)
            t = pers.tile([128, 1], F32, name=f"bks{o}", tag=f"bks{o}")
            nc.sync.dma_start(t[:], bkd[o])
            bk_sb.append(t)
        ones1 = pers.tile([1, 128], BF, tag="ones1")
        nc.vector.memset(ones1[:], 1.0)
        bvr = pers.tile([1, D], BF, tag="bvr")
        nc.sync.dma_start(bvr[:], bvd[:])
        bvb = pers.tile([128, D], F32, tag="bvb")

        # ---- persistent packed-Q / packed-V tiles (double-set by quad
        # parity).  qpk zero regions and vpa/vhb ones columns are written
        # exactly once here; the per-quad adds only touch the data regions.
        qpk = [[pers.tile([128, QUAD * 64], BF, name=f"qpk{s}_{o}", tag=f"qpk{s}_{o}") for o in range(8)]
               for s in range(2)]
        vpa = [[pers.tile([128, 8 * HPW], BF, name=f"vpa{s}_{j}", tag=f"vpa{s}_{j}") for j in range(QUAD)]
               for s in range(2)]
        vhb = [pers.tile([128, 8 * HPW], BF, name=f"vhb{s}", tag=f"vhb{s}") for s in range(2)]
        for s in range(2):
            for o in range(8):
                nc.vector.memset(qpk[s][o][:], 0.0)
            for j in range(QUAD):
                nc.vector.memset(vpa[s][j][:], 1.0)
            nc.vector.memset(vhb[s][:], 1.0)

        # ---- input tiles, DMA'd one quad ahead -------------------------
        xin_cache = {}

        def _quad_inputs(gq):
            if gq in xin_cache:
                return xin_cache[gq]
            c0 = gq * QUAD * L
            cv0 = gq * QUAD * LK
            cq0 = gq * QUAD * LQ
            xt_sb = [xtp.tile([128, QUAD * L], BF, name="xt", tag="xt") for _ in range(8)]
            xq_sb = [xtp.tile([128, QUAD * LQ], BF, name="xq", tag="xq", bufs=24) for _ in range(8)]
            xv_sb = [xtp.tile([128, QUAD * LK], BF, name="xv", tag="xv") for _ in range(8)]
            for d in range(8):
                nc.sync.dma_start(xt_sb[d][:], xt[d * 128 : (d + 1) * 128, c0 : c0 + QUAD * L])
            for d in range(8):
                nc.sync.dma_start(xq_sb[d][:], xqt[d * 128 : (d + 1) * 128, cq0 : cq0 + QUAD * LQ])
            for d in range(8):
                nc.sync.dma_start(xv_sb[d][:], xvt[d * 128 : (d + 1) * 128, cv0 : cv0 + QUAD * LK])
            xin_cache[gq] = (xt_sb, xq_sb, xv_sb)
            return xin_cache[gq]

        # ---- weights: wk, then quad-0 inputs, then wq/wv ---------------
        wq_sb, wk_sb, wv_sb = [], [], []
        for lst, nm in ((wk_sb, "wk"), (wq_sb, "wq"), (wv_sb, "wv")):
            for d in range(8):
                lst.append(pers.tile([128, D], BF, name=f"{nm}{d}", tag=f"{nm}{d}"))
        for d in range(8):
            nc.sync.dma_start(wk_sb[d][:], wkt[d * 128 : (d + 1) * 128, :])
        _quad_inputs(0)
        for lst, src in ((wq_sb, wqt), (wv_sb, wvt)):
            for d in range(8):
                nc.sync.dma_start(lst[d][:], src[d * 128 : (d + 1) * 128, :])

        warm_refs = {}

        def _filler():
            # dense dummy matmul group: keeps HAM un-throttled through
            # attention-only stretches
            ps = pproj.tile([128, 512], F32, name="fill", tag="proj")
            for i in range(3):
                nc.tensor.matmul(ps[:], warm_refs["a"][:], warm_refs["b"][:],
                                 start=(i == 0), stop=(i == 2))
            nc.vector.tensor_copy(warm_refs["o"][:], ps[:])

        def _emit_warmup_bvb():
            # Full-array warm-up matmuls on memset data: keeps the PE busy
            # (and HAM un-throttled) while the weight/input DMAs land.
            wma = pers.tile([128, 128], BF, tag="wma")
            nc.vector.memset(wma[:], 1.0 / 128.0)
            wmb = pers.tile([128, 512], BF, tag="wmb")
            nc.vector.memset(wmb[:], 1.0)
            wmo = pers.tile([128, 512], BF, tag="wmo")
            for g in range(6):
                ps = pproj.tile([128, 512], F32, name="warm", tag="proj")
                for i in range(8):
                    nc.tensor.matmul(ps[:], wma[:], wmb[:], start=(i == 0), stop=(i == 7))
                nc.vector.tensor_copy(wmo[:], ps[:])
            warm_refs.update({"a": wma, "b": wmb, "o": wmo})
            # broadcast bv across partitions via K=1 matmuls with ones
            for oc in range(2):
                ps = pproj.tile([128, 512], F32, name="warm2", tag="proj")
                nc.tensor.matmul(ps[:], ones1[:], bvr[:, oc * 512 : (oc + 1) * 512],
                                 start=True, stop=True)
                nc.vector.tensor_copy(bvb[:, oc * 512 : (oc + 1) * 512], ps[:])

        # ---- per-quad state --------------------------------------------
        ST = {}

        def _quad_state(gq):
            if gq in ST:
                return ST[gq]
            s = {
                "kht": [khtp.tile([128, QUAD * L], BF, name=f"kht{o}", tag=f"kht{o}") for o in range(8)],
                "ma": None, "mb": None,
            }
            t = mapl.tile([LK, QUAD * 4 * LQ], BF, name="ma", tag="ma")
            nc.sync.dma_start(t[:], mad[gq])
            s["ma"] = t
            t = mbpl.tile([QUAD * LQ, 4 * LQ], BF, name="mb", tag="mb")
            nc.sync.dma_start(t[:], mbd[gq])
            s["mb"] = t
            ST[gq] = s
            return s

        # ---- projection work units (one PSUM group each) ---------------
        def proj_units(gq):
            s = _quad_state(gq)
            par = gq % 2
            xt_sb, xq_sb, xv_sb = _quad_inputs(gq)
            units = []

            def k_unit(o, sub):
                def run():
                    ps = pproj.tile([128, 512], F32, name="kps", tag="proj")
                    for d in range(8):
                        nc.tensor.matmul(
                            ps[:, 0:320],
                            wk_sb[d][:, o * 128 : (o + 1) * 128],
                            xt_sb[d][:, sub * 320 : (sub + 1) * 320],
                            start=(d == 0), stop=(d == 7),
                        )
                    nc.vector.tensor_scalar_add(
                        s["kht"][o][:, sub * 320 : (sub + 1) * 320],
                        ps[:, 0:320], bk_sb[o][:],
                    )
                return run

            def q_unit(o):
                def run():
                    ps = pproj.tile([128, 512], F32, name="qps", tag="proj")
                    for d in range(8):
                        nc.tensor.matmul(
                            ps[:, 0 : QUAD * LQ],
                            wq_sb[d][:, o * 128 : (o + 1) * 128],
                            xq_sb[d][:], start=(d == 0), stop=(d == 7),
                        )
                    # scatter into the block-diagonal packed-Q tile:
                    # head-even rows -> col block [0,32), head-odd rows ->
                    # col block [32,64) of each batch's 64-col group.
                    qv = qpk[par][o].rearrange("p (j c) -> p j c", c=64)
                    pv = ps.rearrange("p (j c) -> p j c", c=LQ)
                    nc.vector.tensor_scalar_add(
                        qv[0:64, 0:QUAD, 0:32], pv[0:64, 0:QUAD, :], bq_sb[o][0:64]
                    )
                    nc.vector.tensor_scalar_add(
                        qv[64:128, 0:QUAD, 32:64], pv[64:128, 0:QUAD, :], bq_sb[o][64:128]
                    )
                return run

            def va_unit(j, oc):
                def run():
                    ps = pproj.tile([128, 512], F32, name="vps", tag="proj")
                    for d in range(8):
                        nc.tensor.matmul(
                            ps[:],
                            xv_sb[d][:, j * LK : (j + 1) * LK],
                            wv_sb[d][:, oc * 512 : (oc + 1) * 512],
                            start=(d == 0), stop=(d == 7),
                        )
                    for t in range(4):
                        hp = oc * 4 + t
                        nc.vector.tensor_add(
                            vpa[par][j][:, hp * HPW : hp * HPW + 128],
                            ps[:, t * 128 : (t + 1) * 128],
                            bvb[:, hp * 128 : (hp + 1) * 128],
                        )
                return run

            def vb_unit(oc):
                def run():
                    ps = pproj.tile([128, 512], F32, name="vbps", tag="proj")
                    for d in range(8):
                        nc.tensor.matmul(
                            ps[:], xq_sb[d][:],
                            wv_sb[d][:, oc * 512 : (oc + 1) * 512],
                            start=(d == 0), stop=(d == 7),
                        )
                    for t in range(4):
                        hp = oc * 4 + t
                        nc.vector.tensor_add(
                            vhb[par][:, hp * HPW : hp * HPW + 128],
                            ps[:, t * 128 : (t + 1) * 128],
                            bvb[:, hp * 128 : (hp + 1) * 128],
                        )
                return run

            for o in range(8):
                for sub in range(2):
                    units.append(k_unit(o, sub))
            for o in range(8):
                units.append(q_unit(o))
            for j in range(QUAD):
                for oc in range(2):
                    units.append(va_unit(j, oc))
            for oc in range(2):
                units.append(vb_unit(oc))
            return units

        # ---- attention work units (software-pipelined over head groups) -
        def attn_units(gq):
            s = _quad_state(gq)
            par = gq % 2
            box = {"psa": {}, "psb": {}, "mea": {}, "meb": {}, "psc": {}, "outq": None}

            def a_unit(hg):
                def run():
                    if box["outq"] is None:
                        box["outq"] = outp.tile([128, D], BF, name="outq", tag="outq")
                    pa = psa.tile([128, 512], F32, name="sa", tag="sa")
                    for j in range(QUAD):
                        for p in range(2):
                            o = 2 * hg + p
                            nc.tensor.matmul(
                                pa[:, j * 128 + p * 64 : j * 128 + (p + 1) * 64],
                                s["kht"][o][:, j * L : j * L + LK],
                                qpk[par][o][:, j * 64 : (j + 1) * 64],
                                start=True, stop=True,
                            )
                    box["psa"][hg] = pa
                return run

            def b_unit(hg):
                def run():
                    pb = psb.tile([128, 128], F32, name="sb", tag="sb")
                    for j in range(QUAD):
                        for p in range(2):
                            o = 2 * hg + p
                            nc.tensor.matmul(
                                pb[j * 32 : (j + 1) * 32, p * 64 : (p + 1) * 64],
                                s["kht"][o][:, j * L + LK : (j + 1) * L],
                                qpk[par][o][:, j * 64 : (j + 1) * 64],
                                start=True, stop=True, tile_position=(0, 32 * j),
                            )
                    box["psb"][hg] = pb
                return run

            def e_unit(hg):
                def run():
                    pa = box["psa"].pop(hg)
                    mea = []
                    for j in range(QUAD):
                        m = meap.tile([LK, 4 * LQ], BF, name="mea", tag="mea")
                        nc.scalar.activation(m[:], pa[:, j * 128 : (j + 1) * 128], EXP, scale=SCALE)
                        nc.vector.tensor_mul(m[:], m[:], s["ma"][:, j * 128 : (j + 1) * 128])
                        mea.append(m)
                    box["mea"][hg] = mea
                    pb = box["psb"].pop(hg)
                    meb = mebp.tile([QUAD * LQ, 4 * LQ], BF, name="meb", tag="meb")
                    nc.scalar.activation(meb[:], pb[:], EXP, scale=SCALE)
                    nc.vector.tensor_mul(meb[:], meb[:], s["mb"][:])
                    box["meb"][hg] = meb
                return run

            def c_unit(hg, u):
                # one head pair (heads 4*hg+2*u, 4*hg+2*u+1) x 4 batches,
                # all four batches sharing one PSUM bank via [64,129] slots
                def run():
                    mea = box["mea"][hg]
                    meb = box["meb"][hg]
                    hp = 2 * hg + u
                    pc = pcp.tile([128, 2 * HPW], F32, name="ctx", tag="ctx")
                    for j in range(QUAD):
                        jm, jc = j % 2, j // 2
                        nc.tensor.matmul(
                            pc[64 * jm : 64 * jm + 64, HPW * jc : HPW * (jc + 1)],
                            mea[j][:, u * 64 : (u + 1) * 64],
                            vpa[par][j][:, hp * HPW : (hp + 1) * HPW],
                            start=True, stop=False, tile_position=(0, 64 * jm),
                        )
                        nc.tensor.matmul(
                            pc[64 * jm : 64 * jm + 64, HPW * jc : HPW * (jc + 1)],
                            meb[j * 32 : (j + 1) * 32, u * 64 : (u + 1) * 64],
                            vhb[par][j * 32 : (j + 1) * 32, hp * HPW : (hp + 1) * HPW],
                            start=False, stop=True, tile_position=(32 * j, 64 * jm),
                        )
                    box["psc"][(hg, u)] = pc
                return run

            def d_unit(hg, u):
                def run():
                    pc = box["psc"].pop((hg, u))
                    outq = box["outq"]
                    r = mscp.tile([128, 2], F32, name="r", tag="r")
                    for jc in range(2):
                        nc.vector.tensor_scalar_add(
                            r[:, jc : jc + 1], pc[:, HPW * jc + 128 : HPW * jc + 129], 1e-8
                        )
                    nc.vector.reciprocal(r[:], r[:])
                    for j in range(QUAD):
                        jm, jc = j % 2, j // 2
                        for h2 in range(2):
                            h = 4 * hg + 2 * u + h2
                            nc.scalar.activation(
                                outq[j * 32 : (j + 1) * 32, h * DK : (h + 1) * DK],
                                pc[64 * jm + 32 * h2 : 64 * jm + 32 * h2 + 32,
                                   HPW * jc + 64 * h2 : HPW * jc + 64 * h2 + 64],
                                COPY, scale=r[64 * jm + 32 * h2 : 64 * jm + 32 * h2 + 32, jc : jc + 1],
                            )
                    if (hg, u) == (NHG - 1, 1):
                        nc.sync.dma_start(
                            outd[gq * 128 : (gq + 1) * 128, :], outq[:]
                        )
                return run

            # software pipeline: keep the PE fed (A/B/C) while the scalar
            # and vector engines chew on E/D of earlier head groups.
            us = [a_unit(0), b_unit(0), e_unit(0),
                  a_unit(1), b_unit(1), c_unit(0, 0), c_unit(0, 1), e_unit(1),
                  a_unit(2), b_unit(2), d_unit(0, 0), c_unit(1, 0), d_unit(0, 1),
                  c_unit(1, 1), e_unit(2),
                  a_unit(3), b_unit(3), d_unit(1, 0), c_unit(2, 0), d_unit(1, 1),
                  c_unit(2, 1), e_unit(3),
                  d_unit(2, 0), c_unit(3, 0), d_unit(2, 1), c_unit(3, 1),
                  d_unit(3, 0), d_unit(3, 1)]
            return us

        def _interleave(a, b, pad_every=3):
            na, nb = len(a), len(b)
            ia = ib = 0
            since_b = 0
            while ia < na or ib < nb:
                if ib >= nb or (ia < na and ia * nb <= ib * na):
                    a[ia]()
                    ia += 1
                    since_b += 1
                    if ib >= nb and since_b >= pad_every and ia < na:
                        _filler()
                        since_b = 0
                else:
                    b[ib]()
                    ib += 1
                    since_b = 0

        # ---- emission schedule -----------------------------------------
        _emit_warmup_bvb()
        for u in proj_units(0):
            u()
        for gq in range(NQ):
            if gq + 2 < NQ:
                _quad_inputs(gq + 2)
                _quad_state(gq + 2)
            nxt = proj_units(gq + 1) if gq + 1 < NQ else []
            _interleave(attn_units(gq), nxt)

    nc.compile()
    return nc


def _get_nc():
    global _NC
    if _NC is None:
        _NC = _build()
    return _NC


def kernel(**inputs):
    global _LAST_RESULT
    Q_seq = np.asarray(inputs["Q_seq"], dtype=np.float32)
    K_seq = np.asarray(inputs["K_seq"], dtype=np.float32)
    V_seq = np.asarray(inputs["V_seq"], dtype=np.float32)
    tm = np.asarray(inputs["title_mask"], dtype=np.float32)
    bm = np.asarray(inputs["body_mask"], dtype=np.float32)
    Wq = np.asarray(inputs["Wq"], dtype=np.float32)
    Wk = np.asarray(inputs["Wk"], dtype=np.float32)
    Wv = np.asarray(inputs["Wv"], dtype=np.float32)
    bq = np.asarray(inputs["bq"], dtype=np.float32)
    bk = np.asarray(inputs["bk"], dtype=np.float32)
    bv = np.asarray(inputs["bv"], dtype=np.float32)

    bf = ml_dtypes.bfloat16
    # K_cat = concat(K_seq, Q_seq); V_cat = concat(V_seq, Q_seq). The V
    # projection of the shared Q_seq rows reuses xqt, so xvt only carries
    # the V_seq part.
    Xk = np.concatenate([K_seq, Q_seq], axis=1)  # [B, L, D]

    wqt = np.ascontiguousarray(Wq.T).astype(bf)
    wkt = np.ascontiguousarray(Wk.T).astype(bf)
    wvt = np.ascontiguousarray(Wv.T).astype(bf)

    # body mask [B, 128 keys, 32 q] tiled x4 along cols -> [B, 128, 128]
    maska_bt = (bm * tm[:, :, None]).transpose(0, 2, 1)  # [B,128,32]
    maska4 = np.tile(maska_bt, (1, 1, 4))  # [B, 128, 128]
    # title mask outer product, tiled x4 -> [B, 32, 128]
    maskb = tm[:, :, None] * tm[:, None, :]  # [B, 32(i), 32(q)]
    maskb4 = np.tile(maskb, (1, 1, 4))  # [B, 32, 128]

    nc = _get_nc()
    in_maps = []
    for c in range(NCORES):
        sl = slice(c * NB, (c + 1) * NB)
        XT = np.ascontiguousarray(Xk[sl].reshape(NB * L, D).T).astype(bf)
        XVT = np.ascontiguousarray(V_seq[sl].reshape(NB * LK, D).T).astype(bf)
        XQT = np.ascontiguousarray(Q_seq[sl].reshape(NB * LQ, D).T).astype(bf)
        ma = maska4[sl].reshape(NQ, QUAD, LK, 4 * LQ).transpose(0, 2, 1, 3)
        ma = np.ascontiguousarray(ma.reshape(NQ, LK, QUAD * 4 * LQ)).astype(bf)
        mb = np.ascontiguousarray(maskb4[sl].reshape(NQ, QUAD * LQ, 4 * LQ)).astype(bf)
        in_maps.append({
            "xt": XT,
            "xvt": XVT,
            "xqt": XQT,
            "wqt": wqt, "wkt": wkt, "wvt": wvt,
            "bq": np.ascontiguousarray(bq.reshape(8, 128, 1)),
            "bk": np.ascontiguousarray(bk.reshape(8, 128, 1)),
            "bvrow": np.ascontiguousarray(bv.reshape(1, D)).astype(bf),
            "maska": ma,
            "maskb": mb,
        })

    res = run_bass_kernel_spmd(nc, in_maps, core_ids=list(range(NCORES)))
    _LAST_RESULT = res
    out = np.concatenate(
        [np.asarray(res.results[c]["out"], dtype=np.float32).reshape(NB, LQ, D) for c in range(NCORES)], axis=0
    )
    return np.ascontiguousarray(out.astype(np.float32))
